# revision 17
# baseline (speedup 1.0000x reference)
"""Trainium2 Bass kernel for nn_Conv1dFFTInt8.

The reference computes, per (b, o):
    out[b,o,0] = ifft(fft(x) . fft(w) summed over cin)[0] + bias[o]
By the circular correlation theorem this collapses to a plain dot product:
    out[b,o] = sum_{i,n} x[b,i,n] * w[o,i,(L-n) % L] + bias[o]

So the whole problem is a GEMM: [B, CIN*L] @ [CIN*L, COUT] with a 524288-deep
contraction, sharded over CIN across 8 cores (16 channels each).

impl "dr" (current best): fp8 DoubleRow matmuls. x is split into two fp8e4m3
planes (a = fp8(x), r = fp8(x - a), combined error ~2^-8) packed side by side
in the stationary operand [128, 2, 32] = (k-pair, [a cols | r cols]); w is the
moving operand [128, 2, 128]. One matmul covers a 256-deep contraction at 0.5
cycles/row, so the tensor engine (~6 us) never lags the weight DMA stream
(~25 us at the ~420 GB/s 16-engine pool limit). Weights stream on both HW DGE
queues (sync + scalar) interleaved with x chunks in k-order.

impl "raw": previous single-rate fp16/fp8 version (fallback).
"""

import numpy as np
import ml_dtypes

import concourse.bass as bass
from concourse import bacc
import concourse.mybir as mybir
import concourse.tile as tile
from concourse.bass_utils import run_bass_kernel_spmd

B, CIN, COUT, L = 16, 128, 128, 4096
NCORES = 8
CIN_SH = CIN // NCORES          # 16 channels per core
KT = 128                        # contraction depth per k-tile
NKT = CIN_SH * L // KT          # 512 k-tiles per core
NPAIR = NKT // 2                # 256 DoubleRow k-pairs per core
XW = 2 * B                      # stationary cols: [a(16) | r(16)]

# --- tunables (A/B config) ---
# dma_sched: issue-ordered list of (kind, k0_tile, k1_tile, ring);
# ring 0 = sync HW queue, 1 = scalar HW queue. Entries with the same ring
# form that queue's FIFO. Chunks must be pair-aligned (even tile bounds).
# x is split into 8 small chunks interleaved between w chunks so the two
# queues land w chunks at an even ~2.5us cadence; the last w chunks taper
# so the final matmul burst after the stream ends is short.
# One homogeneous stream per HW queue: interleaving w/x chunks across both
# queues measured ~340 GB/s aggregate vs ~420 GB/s with a solo w stream.
# w rides the scalar queue (which also carries the output DMA while still
# warm — a cold queue adds ~2 us restart lag); x rides sync.
_DR_SCHED = (
    ("x", 0, 256, 0), ("x", 256, 512, 0),
    ("w", 0, 128, 1), ("w", 128, 256, 1), ("w", 256, 352, 1),
    ("w", 352, 416, 1), ("w", 416, 464, 1), ("w", 464, 496, 1),
    ("w", 496, 512, 1),
)
# filler (dummy) matmuls issued before waiting on each w chunk, in k-order
# of the w chunks; keeps the PE p-state ramped across DMA-landing gaps.
_DR_FILL = (0, 0, 0, 0, 0, 0, 0)

CFG = dict(
    impl="dr",                  # "dr" | "raw" | "tile"
    nstrip=4,                   # PSUM accumulation banks, round-robin
    dma_sched=_DR_SCHED,
    fill=_DR_FILL,
    warmup=0,                   # unused in dr (see fill)
    # raw impl knobs (fallback)
    w_dtype="fp8",
    chunks=(16, 48, 64, 128, 128, 128),
    w_sched=tuple((32, 0) for _ in range(16)),
    x_sched=(128, 128, 128, 128),
)

TRACE = False                   # set by test.py to profile
LAST_RESULTS = None             # BassKernelResults of the last run

_PROG_CACHE = {}

FP8 = ml_dtypes.float8_e4m3


def _dt_of(name):
    return {"fp16": (mybir.dt.float16, np.float16),
            "fp8": (mybir.dt.float8e4, FP8)}[name]


def _build_program_dr(cfg):
    """fp8 DoubleRow GEMM with dual-queue DMA."""
    nstrip = cfg["nstrip"]
    sched = cfg["dma_sched"]
    assert all(k0 % 2 == 0 and k1 % 2 == 0 for _, k0, k1, _ in sched)
    w_cover = sorted((k0, k1) for kind, k0, k1, _ in sched if kind == "w")
    x_cover = sorted((k0, k1) for kind, k0, k1, _ in sched if kind == "x")
    for cover in (w_cover, x_cover):
        assert cover[0][0] == 0 and cover[-1][1] == NKT
        assert all(a[1] == b[0] for a, b in zip(cover, cover[1:]))

    fp8 = mybir.dt.float8e4

    nc = bacc.Bacc("TRN2", target_bir_lowering=False, debug=False,
                   num_devices=NCORES)
    xc_d = nc.dram_tensor("xc", [KT, NKT * XW], fp8, kind="ExternalInput")
    wt_d = nc.dram_tensor("wt", [KT, NKT * COUT], fp8, kind="ExternalInput")
    out_d = nc.dram_tensor("out", [XW, nstrip * COUT], mybir.dt.float32,
                           kind="ExternalOutput")

    # chunk sem index to wait before using pair t, per operand kind
    def chunk_of(kind, tile_idx):
        for i, (knd, k0, k1, _r) in enumerate(sched):
            if knd == kind and k0 <= tile_idx < k1:
                return i
        raise AssertionError((kind, tile_idx))

    import contextlib
    with contextlib.ExitStack() as stack:
        ec = stack.enter_context
        s_d = [ec(nc.semaphore(f"s_d{i}")) for i in range(len(sched))]
        s_mm = ec(nc.semaphore("s_mm"))
        s_cp = ec(nc.semaphore("s_cp"))
        s_out = ec(nc.semaphore("s_out"))
        xs = ec(nc.sbuf_tensor("xs", [KT, NKT, XW], fp8))
        ws = ec(nc.sbuf_tensor("ws", [KT, NKT, COUT], fp8))
        # DoubleRow dst must start at partition 0, so the nstrip accumulation
        # banks all use partitions 0..31 and are laid out along osb's free dim
        osb = ec(nc.sbuf_tensor("osb", [KT, nstrip * COUT], mybir.dt.float32))
        accs = [ec(nc.psum_tensor(f"acc{s}", [KT, COUT], mybir.dt.float32))
                for s in range(nstrip)]
        junk = ec(nc.sbuf_tensor("junk", [KT, 2, XW], fp8))
        scr = ec(nc.psum_tensor("scr", [KT, COUT], mybir.dt.float32))

        def emit_dmas(eng, ring):
            for i, (kind, k0, k1, r) in enumerate(sched):
                if r != ring:
                    continue
                if kind == "w":
                    dma = eng.dma_start(ws[:, k0:k1, :],
                                        wt_d[:, k0 * COUT:k1 * COUT])
                else:
                    dma = eng.dma_start(xs[:, k0:k1, :],
                                        xc_d[:, k0 * XW:k1 * XW])
                dma.then_inc(s_d[i], 16)

        with nc.Block() as block:

            @block.sync
            def _(sync):
                emit_dmas(sync, 0)
                sync.wait_ge(s_out, 16)

            @block.scalar
            def _(scalar):
                emit_dmas(scalar, 1)
                scalar.wait_ge(s_cp, 2)
                scalar.dma_start(out_d[:], osb[0:XW, :]).then_inc(s_out, 16)

            @block.tensor
            def _(tensor):
                fill = cfg["fill"]
                w_chunks = sorted(
                    (k0, i) for i, (knd, k0, k1, _r) in enumerate(sched)
                    if knd == "w")
                fill_of = {i: fill[j] for j, (_k0, i) in enumerate(w_chunks)}

                def dummies(n):
                    for _ in range(n):
                        tensor.matmul(
                            scr[0:XW, 0:B], junk[:], junk[:, :, 0:B],
                            start=True, stop=True,
                            perf_mode=mybir.MatmulPerfMode.DoubleRow)

                waited = set()
                for t in range(NPAIR):
                    for need in (chunk_of("w", 2 * t), chunk_of("x", 2 * t),
                                 chunk_of("w", 2 * t + 1),
                                 chunk_of("x", 2 * t + 1)):
                        if need not in waited:
                            waited.add(need)
                            if sched[need][0] == "w":
                                dummies(fill_of.get(need, 0))
                            tensor.wait_ge(s_d[need], 16)
                    s = t % nstrip
                    mm = tensor.matmul(
                        accs[s][0:XW, :],
                        xs[:, 2 * t:2 * t + 2, :],     # lhsT [128, 2, 32]
                        ws[:, 2 * t:2 * t + 2, :],     # rhs  [128, 2, 128]
                        start=(t == s),
                        stop=(t == NPAIR - nstrip + s),
                        perf_mode=mybir.MatmulPerfMode.DoubleRow,
                    )
                    if t == NPAIR - 1:
                        mm.then_inc(s_mm, 1)

            @block.vector
            def _(vector):
                vector.wait_ge(s_mm, 1)
                for s in range(nstrip):
                    cp = vector.tensor_copy(osb[0:XW, s * COUT:(s + 1) * COUT],
                                            accs[s][0:XW, :])
                    if s % 2 == 1:
                        cp.then_inc(s_cp, 1)



    nc.compile()
    return nc


def _build_program(cfg):
    chunks = cfg["chunks"]
    assert sum(chunks) == NKT
    nstrip = cfg["nstrip"]
    w_dt, _ = _dt_of(cfg["w_dtype"])
    x_dt = mybir.dt.float16

    nc = bacc.Bacc("TRN2", target_bir_lowering=False, debug=False,
                   num_devices=NCORES)
    xt_d = nc.dram_tensor("xt", [KT, NKT * B], x_dt, kind="ExternalInput")
    wt_d = nc.dram_tensor("wt", [KT, NKT * COUT], w_dt, kind="ExternalInput")
    out_d = nc.dram_tensor("out", [KT, COUT], mybir.dt.float32,
                           kind="ExternalOutput")

    first_k = {j: j for j in range(nstrip)}
    last_k = {j: NKT - nstrip + j for j in range(nstrip)}
    assert all((last_k[j] % nstrip) == j for j in range(nstrip))

    with tile.TileContext(nc) as tc:
        with tc.tile_pool(name="xp", bufs=len(chunks)) as xp, \
             tc.tile_pool(name="wp", bufs=len(chunks)) as wp, \
             tc.tile_pool(name="pp", bufs=1, space="PSUM") as pp, \
             tc.tile_pool(name="op", bufs=1) as op:
            accs = [pp.tile([KT, COUT], mybir.dt.float32, tag=f"acc{j}",
                            name=f"acc{j}")
                    for j in range(nstrip)]
            k0 = 0
            for c, chunk in enumerate(chunks):
                xc = xp.tile([KT, chunk * B], x_dt, tag="xc")
                nc.scalar.dma_start(
                    xc[:], xt_d[:, k0 * B:(k0 + chunk) * B])
                wc = wp.tile([KT, chunk * COUT], w_dt, tag="wc")
                nc.sync.dma_start(
                    wc[:], wt_d[:, k0 * COUT:(k0 + chunk) * COUT])
                for j in range(chunk):
                    k = k0 + j
                    s = k % nstrip
                    nc.tensor.matmul(
                        accs[s][32 * s:32 * s + B, :],
                        xc[:, j * B:(j + 1) * B],          # lhsT [128, 16]
                        wc[:, j * COUT:(j + 1) * COUT],    # rhs [128, 128]
                        start=(k == first_k[s]),
                        stop=(k == last_k[s]),
                        tile_position=(0, 32 * s),
                    )
                k0 += chunk
            ot = op.tile([KT, COUT], mybir.dt.float32)
            for s in range(nstrip):
                nc.vector.tensor_copy(ot[32 * s:32 * s + B, :],
                                      accs[s][32 * s:32 * s + B, :])
            nc.sync.dma_start(out_d[:], ot[:])
    nc.compile()
    return nc


def _build_program_raw(cfg):
    """Raw bacc implementation: manual semaphores, no TileContext, so the
    multi-microsecond Tile preamble/drain/butterfly disappears."""
    nstrip = cfg["nstrip"]
    w_dt, _ = _dt_of(cfg["w_dtype"])
    x_dt = mybir.dt.float16
    w_sched = cfg["w_sched"]
    x_sched = cfg["x_sched"]
    assert sum(c for c, _ in w_sched) == NKT and sum(x_sched) == NKT
    n_wc = len(w_sched)
    n_xc = len(x_sched)
    w_start = np.cumsum([0] + [c for c, _ in w_sched])  # k-tile offsets
    x_start = np.cumsum([0] + list(x_sched))
    x_need = [int(np.searchsorted(x_start, w_start[c + 1], side="left")) - 1
              for c in range(n_wc)]

    first_k = {j: j for j in range(nstrip)}
    last_k = {j: NKT - nstrip + j for j in range(nstrip)}

    nc = bacc.Bacc("TRN2", target_bir_lowering=False, debug=False,
                   num_devices=NCORES)
    xt_d = nc.dram_tensor("xt", [KT, NKT * B], x_dt, kind="ExternalInput")
    wt_d = nc.dram_tensor("wt", [KT, NKT * COUT], w_dt, kind="ExternalInput")
    out_d = nc.dram_tensor("out", [KT, COUT], mybir.dt.float32,
                           kind="ExternalOutput")

    import contextlib
    with contextlib.ExitStack() as stack:
        ec = stack.enter_context
        s_wc = [ec(nc.semaphore(f"s_w{c}")) for c in range(n_wc)]
        s_xc = [ec(nc.semaphore(f"s_x{c}")) for c in range(n_xc)]
        s_mm = ec(nc.semaphore("s_mm"))
        s_cp = ec(nc.semaphore("s_cp"))
        s_out = ec(nc.semaphore("s_out"))
        xs = ec(nc.sbuf_tensor("xs", [KT, NKT * B], x_dt))
        ws = ec(nc.sbuf_tensor("ws", [KT, NKT * COUT], w_dt))
        osb = ec(nc.sbuf_tensor("osb", [KT, COUT], mybir.dt.float32))
        accs = [ec(nc.psum_tensor(f"acc{s}", [KT, COUT], mybir.dt.float32))
                for s in range(nstrip)]
        if cfg["warmup"]:
            junk = ec(nc.sbuf_tensor("junk", [KT, COUT], x_dt))
            scr = ec(nc.psum_tensor("scr", [KT, COUT], mybir.dt.float32))

        def emit_w(eng, ring):
            for c, (chunk, r) in enumerate(w_sched):
                if r != ring:
                    continue
                a, b = int(w_start[c]) * COUT, int(w_start[c + 1]) * COUT
                eng.dma_start(ws[:, a:b], wt_d[:, a:b]).then_inc(s_wc[c], 16)

        with nc.Block() as block:

            @block.sync
            def _(sync):
                emit_w(sync, 0)
                sync.wait_ge(s_cp, 1)
                sync.dma_start(out_d[:], osb[:]).then_inc(s_out, 16)
                sync.wait_ge(s_out, 16)

            @block.scalar
            def _(scalar):
                for c in range(n_xc):
                    a, b = int(x_start[c]) * B, int(x_start[c + 1]) * B
                    scalar.dma_start(xs[:, a:b],
                                     xt_d[:, a:b]).then_inc(s_xc[c], 16)
                emit_w(scalar, 1)

            @block.tensor
            def _(tensor):
                for _ in range(cfg["warmup"]):
                    tensor.matmul(scr[0:B, :], junk[:, 0:B], junk[:, 0:COUT],
                                  start=True, stop=True)
                x_waited = -1
                for c, (chunk, _r) in enumerate(w_sched):
                    tensor.wait_ge(s_wc[c], 16)
                    if x_need[c] > x_waited:
                        x_waited = x_need[c]
                        tensor.wait_ge(s_xc[x_waited], 16)
                    for j in range(chunk):
                        k = int(w_start[c]) + j
                        s = k % nstrip
                        mm = tensor.matmul(
                            accs[s][32 * s:32 * s + B, :],
                            xs[:, k * B:(k + 1) * B],
                            ws[:, k * COUT:(k + 1) * COUT],
                            start=(k == first_k[s]),
                            stop=(k == last_k[s]),
                            tile_position=(0, 32 * s),
                        )
                        if k == NKT - 1:
                            mm.then_inc(s_mm, 1)

            @block.vector
            def _(vector):
                vector.wait_ge(s_mm, 1)
                for s in range(nstrip):
                    cp = vector.tensor_copy(
                        osb[32 * s:32 * s + B, :],
                        accs[s][32 * s:32 * s + B, :],
                    )
                    if s == nstrip - 1:
                        cp.then_inc(s_cp, 1)

    nc.compile()
    return nc


def _get_program(cfg):
    key = repr(sorted(cfg.items()))
    if key not in _PROG_CACHE:
        impl = cfg.get("impl", "tile")
        if impl == "dr":
            _PROG_CACHE[key] = _build_program_dr(cfg)
        elif impl == "raw":
            _PROG_CACHE[key] = _build_program_raw(cfg)
        else:
            _PROG_CACHE[key] = _build_program(cfg)
    return _PROG_CACHE[key]


def _pack_operand(arr_k_major, ncols, np_dt):
    """[K_total, ncols] contraction-major -> SBUF layout [128, NKT*ncols]
    where sb[p, kt*ncols + c] = arr[kt*128 + p, c]."""
    a = arr_k_major.reshape(NKT, KT, ncols).transpose(1, 0, 2)
    return np.ascontiguousarray(a).reshape(KT, NKT * ncols).astype(np_dt)


def kernel(x, weight, bias):
    import os
    if not TRACE:
        # profiling needs an NTFF hook this image lacks; never trace here
        os.environ["BASS_NEVER_TRACE"] = "1"
    else:
        os.environ.pop("BASS_NEVER_TRACE", None)
    x = np.asarray(x, dtype=np.float32)
    weight = np.asarray(weight, dtype=np.float32)
    bias = np.asarray(bias, dtype=np.float32)

    cfg = dict(CFG)
    nc = _get_program(cfg)
    nstrip = cfg["nstrip"]

    # w_rev[o,i,n] = weight[o,i,(L-n) % L]
    idx = (L - np.arange(L)) % L
    wrev = weight[:, :, idx]

    impl = cfg.get("impl", "tile")
    in_maps = []
    for c in range(NCORES):
        i0 = c * CIN_SH
        wflat = wrev[:, i0:i0 + CIN_SH, :].reshape(COUT, CIN_SH * L)
        xflat = x[:, i0:i0 + CIN_SH, :].reshape(B, CIN_SH * L)
        if impl == "dr":
            wt = _pack_operand(wflat.T, COUT, FP8)
            xk = xflat.T                         # [K, B] float32
            xa = xk.astype(FP8)
            xr = (xk - xa.astype(np.float32)).astype(FP8)
            xa_t = xa.reshape(NKT, KT, B)
            xr_t = xr.reshape(NKT, KT, B)
            xc = np.concatenate([xa_t, xr_t], axis=2)      # [NKT, 128, 32]
            xc = np.ascontiguousarray(
                xc.transpose(1, 0, 2)).reshape(KT, NKT * XW)
            in_maps.append({"xc": xc, "wt": wt})
        else:
            _, w_np_dt = _dt_of(cfg["w_dtype"])
            wt = _pack_operand(wflat.T, COUT, w_np_dt)
            xt = _pack_operand(xflat.T, B, np.float16)
            in_maps.append({"xt": xt, "wt": wt})

    global LAST_RESULTS
    res = run_bass_kernel_spmd(nc, in_maps, core_ids=list(range(NCORES)),
                               trace=TRACE)
    LAST_RESULTS = res

    acc = np.zeros((B, COUT), np.float32)
    for c in range(NCORES):
        o = res.results[c]["out"]
        for s in range(nstrip):
            if impl == "dr":
                blk = o[:, s * COUT:(s + 1) * COUT]
                acc += blk[0:B, :]
                acc += blk[B:XW, :]
            else:
                acc += o[32 * s:32 * s + B, :]
    out = acc + bias[None, :]
    return out[:, :, None].astype(np.float32)


# revision 22
# speedup vs baseline: 1.0549x; 1.0549x over previous
"""Trainium2 Bass kernel for nn_Conv1dFFTInt8.

The reference computes, per (b, o):
    out[b,o,0] = ifft(fft(x) . fft(w) summed over cin)[0] + bias[o]
By the circular correlation theorem this collapses to a plain dot product:
    out[b,o] = sum_{i,n} x[b,i,n] * w[o,i,(L-n) % L] + bias[o]

So the whole problem is a GEMM: [B, CIN*L] @ [CIN*L, COUT] with a 524288-deep
contraction, sharded over CIN across 8 cores (16 channels each).

impl "dr" (current best): fp8 DoubleRow matmuls. x is split into two fp8e4m3
planes (a = fp8(x), r = fp8(x - a), combined error ~2^-8) packed side by side
in the stationary operand [128, 2, 32] = (k-pair, [a cols | r cols]); w is the
moving operand [128, 2, 128]. One matmul covers a 256-deep contraction at 0.5
cycles/row, so the tensor engine (~6 us) never lags the weight DMA stream
(~25 us at the ~420 GB/s 16-engine pool limit). Weights stream on both HW DGE
queues (sync + scalar) interleaved with x chunks in k-order.

impl "raw": previous single-rate fp16/fp8 version (fallback).
"""

import numpy as np
import ml_dtypes

import concourse.bass as bass
from concourse import bacc
import concourse.mybir as mybir
import concourse.tile as tile
from concourse.bass_utils import run_bass_kernel_spmd

B, CIN, COUT, L = 16, 128, 128, 4096
NCORES = 8
CIN_SH = CIN // NCORES          # 16 channels per core
KT = 128                        # contraction depth per k-tile
NKT = CIN_SH * L // KT          # 512 k-tiles per core
NPAIR = NKT // 2                # 256 DoubleRow k-pairs per core
XW = 2 * B                      # stationary cols: [a(16) | r(16)]

# --- tunables (A/B config) ---
# dma_sched: issue-ordered list of (kind, k0_tile, k1_tile, ring);
# ring 0 = sync HW queue, 1 = scalar HW queue. Entries with the same ring
# form that queue's FIFO. Chunks must be pair-aligned (even tile bounds).
# x is split into 8 small chunks interleaved between w chunks so the two
# queues land w chunks at an even ~2.5us cadence; the last w chunks taper
# so the final matmul burst after the stream ends is short.
# One homogeneous stream per HW queue: interleaving w/x chunks across both
# queues measured ~340 GB/s aggregate vs ~420 GB/s with a solo w stream.
# w rides the scalar queue (which also carries the output DMA while still
# warm — a cold queue adds ~2 us restart lag); x rides sync.
_DR_SCHED = (
    ("x", 0, 256, 0), ("x", 256, 512, 0),
    ("w", 0, 128, 1), ("w", 128, 256, 1), ("w", 256, 352, 1),
    ("w", 352, 416, 1), ("w", 416, 464, 1), ("w", 464, 496, 1),
    ("w", 496, 512, 1),
)
# filler (dummy) matmuls issued before waiting on each w chunk, in k-order
# of the w chunks; keeps the PE p-state ramped across DMA-landing gaps.
_DR_FILL = (0, 0, 0, 0, 0, 0, 0)

CFG = dict(
    impl="fwl",                 # "fwl" | "dr" | "raw" | "tile"
    nstrip=4,                   # PSUM accumulation banks, round-robin
    dma_sched=_DR_SCHED,
    fill=_DR_FILL,
    warmup=0,                   # unused in dr (see fill)
    # raw impl knobs (fallback)
    w_dtype="fp8",
    chunks=(16, 48, 64, 128, 128, 128),
    w_sched=tuple((32, 0) for _ in range(16)),
    x_sched=(128, 128, 128, 128),
)

TRACE = False                   # set by test.py to profile
LAST_RESULTS = None             # BassKernelResults of the last run

_PROG_CACHE = {}

FP8 = ml_dtypes.float8_e4m3


def _dt_of(name):
    return {"fp16": (mybir.dt.float16, np.float16),
            "fp8": (mybir.dt.float8e4, FP8)}[name]


def _build_program_fwl(cfg):
    """fp8 GEMM, w stationary (128 cols -> compiler auto-FWL), x moving.

    Per k-tile: LDWEIGHTS w [128,128] fp8 (FWL: 4 fp8/cycle) + MATMUL
    streaming x [128, 32] (= [a|r] precision planes); out [COUT=128, 32]
    accumulated round-robin across nstrip PSUM banks. HW-measured production
    rate for this shape is ~40 ns/MM pair vs ~80 with DoubleRow (which
    disables FWL)."""
    nstrip = cfg["nstrip"]
    sched = cfg["dma_sched"]
    w_cover = sorted((k0, k1) for kind, k0, k1, _r in sched if kind == "w")
    x_cover = sorted((k0, k1) for kind, k0, k1, _r in sched if kind == "x")
    for cover in (w_cover, x_cover):
        assert cover[0][0] == 0 and cover[-1][1] == NKT
        assert all(a[1] == b[0] for a, b in zip(cover, cover[1:]))

    fp8 = mybir.dt.float8e4

    nc = bacc.Bacc("TRN2", target_bir_lowering=False, debug=False,
                   num_devices=NCORES)
    xc_d = nc.dram_tensor("xc", [KT, NKT * XW], fp8, kind="ExternalInput")
    wt_d = nc.dram_tensor("wt", [KT, NKT * COUT], fp8, kind="ExternalInput")
    out_d = nc.dram_tensor("out", [KT, nstrip * XW], mybir.dt.float32,
                           kind="ExternalOutput")

    def chunk_of(kind, tile_idx):
        for i, (knd, k0, k1, _r) in enumerate(sched):
            if knd == kind and k0 <= tile_idx < k1:
                return i
        raise AssertionError((kind, tile_idx))

    import contextlib
    with contextlib.ExitStack() as stack:
        ec = stack.enter_context
        s_d = [ec(nc.semaphore(f"s_d{i}")) for i in range(len(sched))]
        s_mm = ec(nc.semaphore("s_mm"))
        s_cp = ec(nc.semaphore("s_cp"))
        s_out = ec(nc.semaphore("s_out"))
        xs = ec(nc.sbuf_tensor("xs", [KT, NKT, XW], fp8))
        ws = ec(nc.sbuf_tensor("ws", [KT, NKT, COUT], fp8))
        osb = ec(nc.sbuf_tensor("osb", [KT, nstrip * XW], mybir.dt.float32))
        accs = [ec(nc.psum_tensor(f"acc{s}", [KT, XW], mybir.dt.float32))
                for s in range(nstrip)]
        junk = ec(nc.sbuf_tensor("junk", [KT, COUT], fp8))
        scr = ec(nc.psum_tensor("scr", [KT, XW], mybir.dt.float32))

        def emit_dmas(eng, ring):
            for i, (kind, k0, k1, r) in enumerate(sched):
                if r != ring:
                    continue
                if kind == "w":
                    dma = eng.dma_start(ws[:, k0:k1, :],
                                        wt_d[:, k0 * COUT:k1 * COUT])
                else:
                    dma = eng.dma_start(xs[:, k0:k1, :],
                                        xc_d[:, k0 * XW:k1 * XW])
                dma.then_inc(s_d[i], 16)

        with nc.Block() as block:

            @block.sync
            def _(sync):
                emit_dmas(sync, 0)
                sync.wait_ge(s_out, 16)

            @block.scalar
            def _(scalar):
                emit_dmas(scalar, 1)
                scalar.wait_ge(s_cp, 2)
                scalar.dma_start(out_d[:], osb[:]).then_inc(s_out, 16)

            @block.tensor
            def _(tensor):
                fill = cfg["fill"]
                w_chunks = sorted(
                    (k0, i) for i, (knd, k0, k1, _r) in enumerate(sched)
                    if knd == "w")
                fill_of = {i: fill[j] for j, (_k0, i) in enumerate(w_chunks)}

                def dummies(n):
                    for _ in range(n):
                        tensor.matmul(scr[:, :], junk[:], junk[:, 0:XW],
                                      start=True, stop=True)

                waited = set()
                for t in range(NKT):
                    for need in (chunk_of("w", t), chunk_of("x", t)):
                        if need not in waited:
                            waited.add(need)
                            if sched[need][0] == "w":
                                dummies(fill_of.get(need, 0))
                            tensor.wait_ge(s_d[need], 16)
                    s = t % nstrip
                    mm = tensor.matmul(
                        accs[s][:, :],
                        ws[:, t, :],                   # lhsT [128, 128] w
                        xs[:, t, :],                   # rhs  [128, 32] x a|r
                        start=(t == s),
                        stop=(t == NKT - nstrip + s),
                    )
                    if t == NKT - 1:
                        mm.then_inc(s_mm, 1)

            @block.vector
            def _(vector):
                vector.wait_ge(s_mm, 1)
                for s in range(nstrip):
                    cp = vector.tensor_copy(osb[:, s * XW:(s + 1) * XW],
                                            accs[s][:, :])
                    if s % 2 == 1:
                        cp.then_inc(s_cp, 1)

    nc.compile()
    return nc


def _build_program_dr(cfg):
    """fp8 DoubleRow GEMM with dual-queue DMA."""
    nstrip = cfg["nstrip"]
    sched = cfg["dma_sched"]
    assert all(k0 % 2 == 0 and k1 % 2 == 0 for _, k0, k1, _ in sched)
    w_cover = sorted((k0, k1) for kind, k0, k1, _ in sched if kind == "w")
    x_cover = sorted((k0, k1) for kind, k0, k1, _ in sched if kind == "x")
    for cover in (w_cover, x_cover):
        assert cover[0][0] == 0 and cover[-1][1] == NKT
        assert all(a[1] == b[0] for a, b in zip(cover, cover[1:]))

    fp8 = mybir.dt.float8e4

    nc = bacc.Bacc("TRN2", target_bir_lowering=False, debug=False,
                   num_devices=NCORES)
    xc_d = nc.dram_tensor("xc", [KT, NKT * XW], fp8, kind="ExternalInput")
    wt_d = nc.dram_tensor("wt", [KT, NKT * COUT], fp8, kind="ExternalInput")
    out_d = nc.dram_tensor("out", [XW, nstrip * COUT], mybir.dt.float32,
                           kind="ExternalOutput")

    # chunk sem index to wait before using pair t, per operand kind
    def chunk_of(kind, tile_idx):
        for i, (knd, k0, k1, _r) in enumerate(sched):
            if knd == kind and k0 <= tile_idx < k1:
                return i
        raise AssertionError((kind, tile_idx))

    import contextlib
    with contextlib.ExitStack() as stack:
        ec = stack.enter_context
        s_d = [ec(nc.semaphore(f"s_d{i}")) for i in range(len(sched))]
        s_mm = ec(nc.semaphore("s_mm"))
        s_cp = ec(nc.semaphore("s_cp"))
        s_out = ec(nc.semaphore("s_out"))
        xs = ec(nc.sbuf_tensor("xs", [KT, NKT, XW], fp8))
        ws = ec(nc.sbuf_tensor("ws", [KT, NKT, COUT], fp8))
        # DoubleRow dst must start at partition 0, so the nstrip accumulation
        # banks all use partitions 0..31 and are laid out along osb's free dim
        osb = ec(nc.sbuf_tensor("osb", [KT, nstrip * COUT], mybir.dt.float32))
        accs = [ec(nc.psum_tensor(f"acc{s}", [KT, COUT], mybir.dt.float32))
                for s in range(nstrip)]
        junk = ec(nc.sbuf_tensor("junk", [KT, 2, XW], fp8))
        scr = ec(nc.psum_tensor("scr", [KT, COUT], mybir.dt.float32))

        def emit_dmas(eng, ring):
            for i, (kind, k0, k1, r) in enumerate(sched):
                if r != ring:
                    continue
                if kind == "w":
                    dma = eng.dma_start(ws[:, k0:k1, :],
                                        wt_d[:, k0 * COUT:k1 * COUT])
                else:
                    dma = eng.dma_start(xs[:, k0:k1, :],
                                        xc_d[:, k0 * XW:k1 * XW])
                dma.then_inc(s_d[i], 16)

        with nc.Block() as block:

            @block.sync
            def _(sync):
                emit_dmas(sync, 0)
                sync.wait_ge(s_out, 16)

            @block.scalar
            def _(scalar):
                emit_dmas(scalar, 1)
                scalar.wait_ge(s_cp, 2)
                scalar.dma_start(out_d[:], osb[0:XW, :]).then_inc(s_out, 16)

            @block.tensor
            def _(tensor):
                fill = cfg["fill"]
                w_chunks = sorted(
                    (k0, i) for i, (knd, k0, k1, _r) in enumerate(sched)
                    if knd == "w")
                fill_of = {i: fill[j] for j, (_k0, i) in enumerate(w_chunks)}

                def dummies(n):
                    for _ in range(n):
                        tensor.matmul(
                            scr[0:XW, 0:B], junk[:], junk[:, :, 0:B],
                            start=True, stop=True,
                            perf_mode=mybir.MatmulPerfMode.DoubleRow)

                waited = set()
                for t in range(NPAIR):
                    for need in (chunk_of("w", 2 * t), chunk_of("x", 2 * t),
                                 chunk_of("w", 2 * t + 1),
                                 chunk_of("x", 2 * t + 1)):
                        if need not in waited:
                            waited.add(need)
                            if sched[need][0] == "w":
                                dummies(fill_of.get(need, 0))
                            tensor.wait_ge(s_d[need], 16)
                    s = t % nstrip
                    mm = tensor.matmul(
                        accs[s][0:XW, :],
                        xs[:, 2 * t:2 * t + 2, :],     # lhsT [128, 2, 32]
                        ws[:, 2 * t:2 * t + 2, :],     # rhs  [128, 2, 128]
                        start=(t == s),
                        stop=(t == NPAIR - nstrip + s),
                        perf_mode=mybir.MatmulPerfMode.DoubleRow,
                    )
                    if t == NPAIR - 1:
                        mm.then_inc(s_mm, 1)

            @block.vector
            def _(vector):
                vector.wait_ge(s_mm, 1)
                for s in range(nstrip):
                    cp = vector.tensor_copy(osb[0:XW, s * COUT:(s + 1) * COUT],
                                            accs[s][0:XW, :])
                    if s % 2 == 1:
                        cp.then_inc(s_cp, 1)



    nc.compile()
    return nc


def _build_program(cfg):
    chunks = cfg["chunks"]
    assert sum(chunks) == NKT
    nstrip = cfg["nstrip"]
    w_dt, _ = _dt_of(cfg["w_dtype"])
    x_dt = mybir.dt.float16

    nc = bacc.Bacc("TRN2", target_bir_lowering=False, debug=False,
                   num_devices=NCORES)
    xt_d = nc.dram_tensor("xt", [KT, NKT * B], x_dt, kind="ExternalInput")
    wt_d = nc.dram_tensor("wt", [KT, NKT * COUT], w_dt, kind="ExternalInput")
    out_d = nc.dram_tensor("out", [KT, COUT], mybir.dt.float32,
                           kind="ExternalOutput")

    first_k = {j: j for j in range(nstrip)}
    last_k = {j: NKT - nstrip + j for j in range(nstrip)}
    assert all((last_k[j] % nstrip) == j for j in range(nstrip))

    with tile.TileContext(nc) as tc:
        with tc.tile_pool(name="xp", bufs=len(chunks)) as xp, \
             tc.tile_pool(name="wp", bufs=len(chunks)) as wp, \
             tc.tile_pool(name="pp", bufs=1, space="PSUM") as pp, \
             tc.tile_pool(name="op", bufs=1) as op:
            accs = [pp.tile([KT, COUT], mybir.dt.float32, tag=f"acc{j}",
                            name=f"acc{j}")
                    for j in range(nstrip)]
            k0 = 0
            for c, chunk in enumerate(chunks):
                xc = xp.tile([KT, chunk * B], x_dt, tag="xc")
                nc.scalar.dma_start(
                    xc[:], xt_d[:, k0 * B:(k0 + chunk) * B])
                wc = wp.tile([KT, chunk * COUT], w_dt, tag="wc")
                nc.sync.dma_start(
                    wc[:], wt_d[:, k0 * COUT:(k0 + chunk) * COUT])
                for j in range(chunk):
                    k = k0 + j
                    s = k % nstrip
                    nc.tensor.matmul(
                        accs[s][32 * s:32 * s + B, :],
                        xc[:, j * B:(j + 1) * B],          # lhsT [128, 16]
                        wc[:, j * COUT:(j + 1) * COUT],    # rhs [128, 128]
                        start=(k == first_k[s]),
                        stop=(k == last_k[s]),
                        tile_position=(0, 32 * s),
                    )
                k0 += chunk
            ot = op.tile([KT, COUT], mybir.dt.float32)
            for s in range(nstrip):
                nc.vector.tensor_copy(ot[32 * s:32 * s + B, :],
                                      accs[s][32 * s:32 * s + B, :])
            nc.sync.dma_start(out_d[:], ot[:])
    nc.compile()
    return nc


def _build_program_raw(cfg):
    """Raw bacc implementation: manual semaphores, no TileContext, so the
    multi-microsecond Tile preamble/drain/butterfly disappears."""
    nstrip = cfg["nstrip"]
    w_dt, _ = _dt_of(cfg["w_dtype"])
    x_dt = mybir.dt.float16
    w_sched = cfg["w_sched"]
    x_sched = cfg["x_sched"]
    assert sum(c for c, _ in w_sched) == NKT and sum(x_sched) == NKT
    n_wc = len(w_sched)
    n_xc = len(x_sched)
    w_start = np.cumsum([0] + [c for c, _ in w_sched])  # k-tile offsets
    x_start = np.cumsum([0] + list(x_sched))
    x_need = [int(np.searchsorted(x_start, w_start[c + 1], side="left")) - 1
              for c in range(n_wc)]

    first_k = {j: j for j in range(nstrip)}
    last_k = {j: NKT - nstrip + j for j in range(nstrip)}

    nc = bacc.Bacc("TRN2", target_bir_lowering=False, debug=False,
                   num_devices=NCORES)
    xt_d = nc.dram_tensor("xt", [KT, NKT * B], x_dt, kind="ExternalInput")
    wt_d = nc.dram_tensor("wt", [KT, NKT * COUT], w_dt, kind="ExternalInput")
    out_d = nc.dram_tensor("out", [KT, COUT], mybir.dt.float32,
                           kind="ExternalOutput")

    import contextlib
    with contextlib.ExitStack() as stack:
        ec = stack.enter_context
        s_wc = [ec(nc.semaphore(f"s_w{c}")) for c in range(n_wc)]
        s_xc = [ec(nc.semaphore(f"s_x{c}")) for c in range(n_xc)]
        s_mm = ec(nc.semaphore("s_mm"))
        s_cp = ec(nc.semaphore("s_cp"))
        s_out = ec(nc.semaphore("s_out"))
        xs = ec(nc.sbuf_tensor("xs", [KT, NKT * B], x_dt))
        ws = ec(nc.sbuf_tensor("ws", [KT, NKT * COUT], w_dt))
        osb = ec(nc.sbuf_tensor("osb", [KT, COUT], mybir.dt.float32))
        accs = [ec(nc.psum_tensor(f"acc{s}", [KT, COUT], mybir.dt.float32))
                for s in range(nstrip)]
        if cfg["warmup"]:
            junk = ec(nc.sbuf_tensor("junk", [KT, COUT], x_dt))
            scr = ec(nc.psum_tensor("scr", [KT, COUT], mybir.dt.float32))

        def emit_w(eng, ring):
            for c, (chunk, r) in enumerate(w_sched):
                if r != ring:
                    continue
                a, b = int(w_start[c]) * COUT, int(w_start[c + 1]) * COUT
                eng.dma_start(ws[:, a:b], wt_d[:, a:b]).then_inc(s_wc[c], 16)

        with nc.Block() as block:

            @block.sync
            def _(sync):
                emit_w(sync, 0)
                sync.wait_ge(s_cp, 1)
                sync.dma_start(out_d[:], osb[:]).then_inc(s_out, 16)
                sync.wait_ge(s_out, 16)

            @block.scalar
            def _(scalar):
                for c in range(n_xc):
                    a, b = int(x_start[c]) * B, int(x_start[c + 1]) * B
                    scalar.dma_start(xs[:, a:b],
                                     xt_d[:, a:b]).then_inc(s_xc[c], 16)
                emit_w(scalar, 1)

            @block.tensor
            def _(tensor):
                for _ in range(cfg["warmup"]):
                    tensor.matmul(scr[0:B, :], junk[:, 0:B], junk[:, 0:COUT],
                                  start=True, stop=True)
                x_waited = -1
                for c, (chunk, _r) in enumerate(w_sched):
                    tensor.wait_ge(s_wc[c], 16)
                    if x_need[c] > x_waited:
                        x_waited = x_need[c]
                        tensor.wait_ge(s_xc[x_waited], 16)
                    for j in range(chunk):
                        k = int(w_start[c]) + j
                        s = k % nstrip
                        mm = tensor.matmul(
                            accs[s][32 * s:32 * s + B, :],
                            xs[:, k * B:(k + 1) * B],
                            ws[:, k * COUT:(k + 1) * COUT],
                            start=(k == first_k[s]),
                            stop=(k == last_k[s]),
                            tile_position=(0, 32 * s),
                        )
                        if k == NKT - 1:
                            mm.then_inc(s_mm, 1)

            @block.vector
            def _(vector):
                vector.wait_ge(s_mm, 1)
                for s in range(nstrip):
                    cp = vector.tensor_copy(
                        osb[32 * s:32 * s + B, :],
                        accs[s][32 * s:32 * s + B, :],
                    )
                    if s == nstrip - 1:
                        cp.then_inc(s_cp, 1)

    nc.compile()
    return nc


def _get_program(cfg):
    key = repr(sorted(cfg.items()))
    if key not in _PROG_CACHE:
        impl = cfg.get("impl", "tile")
        if impl == "fwl":
            _PROG_CACHE[key] = _build_program_fwl(cfg)
        elif impl == "dr":
            _PROG_CACHE[key] = _build_program_dr(cfg)
        elif impl == "raw":
            _PROG_CACHE[key] = _build_program_raw(cfg)
        else:
            _PROG_CACHE[key] = _build_program(cfg)
    return _PROG_CACHE[key]


def _pack_operand(arr_k_major, ncols, np_dt):
    """[K_total, ncols] contraction-major -> SBUF layout [128, NKT*ncols]
    where sb[p, kt*ncols + c] = arr[kt*128 + p, c]."""
    a = arr_k_major.reshape(NKT, KT, ncols).transpose(1, 0, 2)
    return np.ascontiguousarray(a).reshape(KT, NKT * ncols).astype(np_dt)


def kernel(x, weight, bias):
    import os
    if not TRACE:
        # profiling needs an NTFF hook this image lacks; never trace here
        os.environ["BASS_NEVER_TRACE"] = "1"
    else:
        os.environ.pop("BASS_NEVER_TRACE", None)
    x = np.asarray(x, dtype=np.float32)
    weight = np.asarray(weight, dtype=np.float32)
    bias = np.asarray(bias, dtype=np.float32)

    cfg = dict(CFG)
    nc = _get_program(cfg)
    nstrip = cfg["nstrip"]

    # w_rev[o,i,n] = weight[o,i,(L-n) % L]
    idx = (L - np.arange(L)) % L
    wrev = weight[:, :, idx]

    impl = cfg.get("impl", "tile")
    in_maps = []
    for c in range(NCORES):
        i0 = c * CIN_SH
        wflat = wrev[:, i0:i0 + CIN_SH, :].reshape(COUT, CIN_SH * L)
        xflat = x[:, i0:i0 + CIN_SH, :].reshape(B, CIN_SH * L)
        if impl in ("dr", "fwl"):
            wt = _pack_operand(wflat.T, COUT, FP8)
            xk = xflat.T                         # [K, B] float32
            xa = xk.astype(FP8)
            xr = (xk - xa.astype(np.float32)).astype(FP8)
            xa_t = xa.reshape(NKT, KT, B)
            xr_t = xr.reshape(NKT, KT, B)
            xc = np.concatenate([xa_t, xr_t], axis=2)      # [NKT, 128, 32]
            xc = np.ascontiguousarray(
                xc.transpose(1, 0, 2)).reshape(KT, NKT * XW)
            in_maps.append({"xc": xc, "wt": wt})
        else:
            _, w_np_dt = _dt_of(cfg["w_dtype"])
            wt = _pack_operand(wflat.T, COUT, w_np_dt)
            xt = _pack_operand(xflat.T, B, np.float16)
            in_maps.append({"xt": xt, "wt": wt})

    global LAST_RESULTS
    res = run_bass_kernel_spmd(nc, in_maps, core_ids=list(range(NCORES)),
                               trace=TRACE)
    LAST_RESULTS = res

    acc = np.zeros((B, COUT), np.float32)
    for c in range(NCORES):
        o = res.results[c]["out"]
        for s in range(nstrip):
            if impl == "fwl":
                blk = o[:, s * XW:(s + 1) * XW]    # [COUT, a|r x B]
                acc += blk[:, 0:B].T
                acc += blk[:, B:XW].T
            elif impl == "dr":
                blk = o[:, s * COUT:(s + 1) * COUT]
                acc += blk[0:B, :]
                acc += blk[B:XW, :]
            else:
                acc += o[32 * s:32 * s + B, :]
    out = acc + bias[None, :]
    return out[:, :, None].astype(np.float32)
